# revision 25
# baseline (speedup 1.0000x reference)
"""Chamfer criterion kernel for Trainium2 (8 NeuronCores, SPMD data-parallel).

Strategy:
  - Shard batch B=32 across 8 cores (4 batches/core).
  - Per batch, compute the [2048, 2048] squared-distance matrix in 128-row
    tiles on the TensorEngine via the gram trick, d[n,m] = sum_k L[k,n]*R[k,m],
    with K=16 bf16 rows: each fp32 coordinate is split hi+lo into two bf16
    values (x ~= ax + bx exactly to 2^-16), so every product is exact in bf16
    and the fp32 PSUM accumulation reconstructs near-fp32 distances at full
    PE rate (1 col/cycle; fp32 matmul would be 4x slower and fp32r's fused
    self-loading form only admits one sync wait in walrus codegen).
  - ScalarE converts each PSUM fp32 tile to SBUF fp16.
  - VectorE: fused tensor_tensor_reduce gives per-row min (min over m);
    a running tensor_tensor(min) accumulates the column-min across n-tiles.
  - Host: final partition-axis min for min2, masking, fp64 sums, KL terms.
"""

import os
import sys

import numpy as np

for _p in ("/opt/trn_rl_repo",):
    if os.path.isdir(_p) and _p not in sys.path:
        sys.path.insert(0, _p)

B, N, D, L = 32, 2048, 3, 5
NCORES = 8
BPC = B // NCORES  # batches per core
# The reference fills padded x with +1e4 and padded y with -1e4 so padded
# points never win a min. Any fill with (pad vs valid) distance > max
# (valid vs valid) distance is equivalent after masking; +/-50 keeps every
# pairwise distance finite in fp16 ((2*50)^2*3 = 30000 < 65504) while
# (50-|c|)^2*3 >= ~3600 still dominates valid distances <= (2*|c|)^2*3.
# Requires max |coord| <= ~14; randn data is ~5.
FILL = 50.0
NT = N // 128  # n-tiles per batch
MCH = 512  # matmul free-dim chunk (one PSUM bank)
MC = N // MCH
K = 16  # gram-trick contraction depth (bf16 hi/lo split rows)

BETA = 0.01
Z_DIM = 16
Z_SCALES = np.array([1, 2, 4, 8, 16], dtype=np.float32)

_CACHE = {}


def _build_program():
    from contextlib import ExitStack

    import concourse.bacc as bacc
    import concourse.mybir as mybir
    import concourse.tile as tile

    f32 = mybir.dt.float32
    bf16 = mybir.dt.bfloat16
    f16 = mybir.dt.float16
    amin = mybir.AluOpType.min

    nc = bacc.Bacc("TRN2", debug=False, enable_asserts=False, num_devices=NCORES)
    # L rows and R rows side by side per K-row: lr[b, k, 0:N]=L, lr[b, k, N:2N]=R
    lr = nc.dram_tensor("lr", [BPC, K, 2 * N], bf16, kind="ExternalInput").ap()
    rowmins = nc.dram_tensor("rowmins", [BPC, 128, NT], f32, kind="ExternalOutput").ap()
    colmin = nc.dram_tensor("colmin", [BPC, 128, N], f16, kind="ExternalOutput").ap()

    with ExitStack() as ctx:
        tc = ctx.enter_context(tile.TileContext(nc))
        in_pool = ctx.enter_context(tc.tile_pool(name="inp", bufs=2))
        psum_pool = ctx.enter_context(tc.tile_pool(name="psum", bufs=2, space="PSUM"))
        d_pool = ctx.enter_context(tc.tile_pool(name="d16", bufs=3))
        scr_pool = ctx.enter_context(tc.tile_pool(name="scr", bufs=2))
        acc_pool = ctx.enter_context(tc.tile_pool(name="acc", bufs=3))
        rm_pool = ctx.enter_context(tc.tile_pool(name="rm", bufs=2))

        for b in range(BPC):
            lr_sb = in_pool.tile([K, 2 * N], bf16, tag="lr")
            nc.sync.dma_start(lr_sb[:], lr[b])
            rm = rm_pool.tile([128, NT], f32, tag="rm")
            acc = None
            for nt in range(NT):
                ps = psum_pool.tile([128, N], f32, tag="ps")
                for mc in range(MC):
                    nc.tensor.matmul(
                        ps[:, mc * MCH : (mc + 1) * MCH],
                        lr_sb[:, nt * 128 : (nt + 1) * 128],
                        lr_sb[:, N + mc * MCH : N + (mc + 1) * MCH],
                        start=True,
                        stop=True,
                    )
                d16 = d_pool.tile([128, N], f16, tag="d16")
                nc.scalar.copy(d16[:], ps[:])
                # Fused: out = min(d16, d16) (= copy), accum_out = row-min.
                # At nt==0 the dead copy doubles as the col-min init.
                if nt == 0:
                    out_tile = acc_pool.tile([128, N], f16, tag="acc", name="acc0")
                else:
                    out_tile = scr_pool.tile([128, N], f16, tag="scr", name="scr")
                if os.environ.get("KVAR", "ttr") == "ttr":
                    nc.vector.tensor_tensor_reduce(
                        out=out_tile[:],
                        in0=d16[:],
                        in1=d16[:],
                        scale=1.0,
                        scalar=3.0e38,
                        op0=amin,
                        op1=amin,
                        accum_out=rm[:, nt : nt + 1],
                    )
                else:
                    nc.vector.tensor_copy(out_tile[:], d16[:])
                    nc.vector.tensor_reduce(
                        rm[:, nt : nt + 1], d16[:], axis=mybir.AxisListType.X, op=amin
                    )
                if nt == 0:
                    acc = out_tile
                else:
                    acc_new = acc_pool.tile([128, N], f16, tag="acc")
                    nc.vector.tensor_tensor(acc_new[:], acc[:], d16[:], amin)
                    acc = acc_new
            nc.sync.dma_start(rowmins[b], rm[:])
            nc.sync.dma_start(colmin[b], acc[:])
    nc.compile()
    return nc


def _get_program():
    if "nc" not in _CACHE:
        _CACHE["nc"] = _build_program()
    return _CACHE["nc"]


def _host_prep(output_set, output_mask, target_set, target_mask):
    """Build the K=16 bf16 hi/lo-split gram-trick operands.

    d[n,m] = x2[n] + y2[m] - 2*x.y, with every term expressed as a sum of
    exact bf16 x bf16 products (accumulated in fp32 PSUM):
      rows 0-2 : (-2*ax_c) * ay_c     rows 3-5 : (-2*ax_c) * by_c
      rows 6-8 : (-2*bx_c) * ay_c     rows 9-11: (-2*bx_c) * by_c
      row 12: x2h*1   row 13: x2l*1   row 14: 1*y2h   row 15: 1*y2l
    where ax=bf16(x), bx=bf16(x-ax) (residual ~2^-16), same for y/x2/y2.
    """
    import ml_dtypes

    bf = ml_dtypes.bfloat16
    maxabs = max(
        np.abs(output_set[~output_mask]).max(initial=0.0),
        np.abs(target_set[~target_mask]).max(initial=0.0),
    )
    assert maxabs <= 14.0, f"coordinate magnitude {maxabs} too large for fp16 fill"
    x = np.where(output_mask[..., None], np.float32(FILL), output_set.astype(np.float32))
    y = np.where(target_mask[..., None], np.float32(-FILL), target_set.astype(np.float32))
    x2 = np.sum(x * x, axis=-1, dtype=np.float32)  # [B, N]
    y2 = np.sum(y * y, axis=-1, dtype=np.float32)

    def split(v):
        hi = v.astype(bf)
        lo = (v - hi.astype(np.float32)).astype(bf)
        return hi.astype(np.float32), lo.astype(np.float32)

    ax, bx = split(x)  # [B, N, 3] each
    ay, by = split(y)
    x2h, x2l = split(x2)  # [B, N]
    y2h, y2l = split(y2)
    ones = np.ones_like(x2)
    zeros = np.zeros_like(x2)

    def rows(*rs):
        return np.stack(rs, axis=1)  # [B, K, N]

    lx = rows(
        *(-2.0 * ax[..., c] for c in range(3)),
        *(-2.0 * ax[..., c] for c in range(3)),
        *(-2.0 * bx[..., c] for c in range(3)),
        *(-2.0 * bx[..., c] for c in range(3)),
        x2h, x2l, ones, ones,
    )
    ry = rows(
        *(ay[..., c] for c in range(3)),
        *(by[..., c] for c in range(3)),
        *(ay[..., c] for c in range(3)),
        *(by[..., c] for c in range(3)),
        ones, ones, y2h, y2l,
    )
    lr = np.concatenate([lx, ry], axis=2)  # [B, K, 2N]
    # -2*ax etc. are exact in bf16 (sign/exponent ops); x2h/ones exact too.
    return np.ascontiguousarray(lr.astype(bf))


def _finalize(rowmins, colmin, output_mask, target_mask, kls):
    # rowmins: [B, 128, NT] f32 with min1[n = nt*128 + p] = rowmins[b, p, nt]
    min1 = rowmins.transpose(0, 2, 1).reshape(B, N).astype(np.float64)
    min2 = colmin.astype(np.float32).min(axis=1).astype(np.float64)  # [B, N]
    validx = ~output_mask
    validy = ~target_mask
    nx = validx.sum(axis=1).astype(np.float64)
    ny = validy.sum(axis=1).astype(np.float64)
    s1 = np.where(validx, min1, 0.0).sum(axis=1)
    s2 = np.where(validy, min2, 0.0).sum(axis=1)
    with np.errstate(divide="ignore", invalid="ignore"):
        per_sample = s1 / nx + s2 / ny
    l2_loss = np.float32(per_sample.mean())

    kl64 = kls.astype(np.float64)
    kl_loss = np.float32(kl64.sum(axis=1).mean())
    loss = np.float32(np.float32(BETA) * kl_loss + l2_loss)
    topdown_kl = (kl64.mean(axis=0) / (Z_SCALES.astype(np.float64) * Z_DIM)).astype(
        np.float32
    )
    beta = np.float32(BETA)
    return loss, kl_loss, l2_loss, topdown_kl, beta


def _run_device(lr, trace=False):
    import concourse.bass_utils as bass_utils

    nc = _get_program()
    in_maps = [{"lr": lr[c * BPC : (c + 1) * BPC]} for c in range(NCORES)]
    res = bass_utils.run_bass_kernel_spmd(
        nc, in_maps, core_ids=list(range(NCORES)), trace=trace
    )
    rowmins = np.concatenate([r["rowmins"] for r in res.results], axis=0)
    colmin = np.concatenate([r["colmin"] for r in res.results], axis=0)
    return rowmins.reshape(B, 128, NT), colmin.reshape(B, 128, N), res


def kernel(output_set, output_mask, target_set, target_mask, kls):
    output_set = np.asarray(output_set)
    output_mask = np.asarray(output_mask).astype(bool)
    target_set = np.asarray(target_set)
    target_mask = np.asarray(target_mask).astype(bool)
    kls = np.asarray(kls)
    assert output_set.shape == (B, N, D), output_set.shape
    lr = _host_prep(output_set, output_mask, target_set, target_mask)
    rowmins, colmin, _ = _run_device(lr)
    return _finalize(rowmins, colmin, output_mask, target_mask, kls)


# revision 26
# speedup vs baseline: 3.6472x; 3.6472x over previous
"""Chamfer criterion kernel for Trainium2 (8 NeuronCores, SPMD data-parallel).

Strategy:
  - Shard batch B=32 across 8 cores (4 batches/core).
  - Per batch, compute the [2048, 2048] squared-distance matrix in 128-row
    tiles on the TensorEngine via the gram trick, d[n,m] = sum_k L[k,n]*R[k,m],
    with K=16 bf16 rows: each fp32 coordinate is split hi+lo into two bf16
    values (x ~= ax + bx exactly to 2^-16), so every product is exact in bf16
    and the fp32 PSUM accumulation reconstructs near-fp32 distances at full
    PE rate (1 col/cycle; fp32 matmul would be 4x slower and fp32r's fused
    self-loading form only admits one sync wait in walrus codegen).
  - ScalarE converts each PSUM fp32 tile to SBUF fp16.
  - VectorE: fused tensor_tensor_reduce gives per-row min (min over m);
    a running tensor_tensor(min) accumulates the column-min across n-tiles.
  - Host: final partition-axis min for min2, masking, fp64 sums, KL terms.
"""

import os
import sys

import numpy as np

for _p in ("/opt/trn_rl_repo",):
    if os.path.isdir(_p) and _p not in sys.path:
        sys.path.insert(0, _p)

B, N, D, L = 32, 2048, 3, 5
NCORES = 8
BPC = B // NCORES  # batches per core
# The reference fills padded x with +1e4 and padded y with -1e4 so padded
# points never win a min. Any fill with (pad vs valid) distance > max
# (valid vs valid) distance is equivalent after masking; +/-50 keeps every
# pairwise distance finite in fp16 ((2*50)^2*3 = 30000 < 65504) while
# (50-|c|)^2*3 >= ~3600 still dominates valid distances <= (2*|c|)^2*3.
# Requires max |coord| <= ~14; randn data is ~5.
FILL = 50.0
NT = N // 128  # n-tiles per batch
MCH = 512  # matmul free-dim chunk (one PSUM bank)
MC = N // MCH
K = 16  # gram-trick contraction depth (bf16 hi/lo split rows)

BETA = 0.01
Z_DIM = 16
Z_SCALES = np.array([1, 2, 4, 8, 16], dtype=np.float32)

_CACHE = {}


def _build_program():
    from contextlib import ExitStack

    import concourse.bacc as bacc
    import concourse.mybir as mybir
    import concourse.tile as tile

    f32 = mybir.dt.float32
    bf16 = mybir.dt.bfloat16
    f16 = mybir.dt.float16
    amin = mybir.AluOpType.min

    nc = bacc.Bacc("TRN2", debug=False, enable_asserts=False, num_devices=NCORES)
    # L rows and R rows side by side per K-row: lr[b, k, 0:N]=L, lr[b, k, N:2N]=R
    lr = nc.dram_tensor("lr", [BPC, K, 2 * N], bf16, kind="ExternalInput").ap()
    rowmins = nc.dram_tensor("rowmins", [BPC, 128, NT], f32, kind="ExternalOutput").ap()
    colmin = nc.dram_tensor("colmin", [BPC, 128, N], f16, kind="ExternalOutput").ap()

    with ExitStack() as ctx:
        tc = ctx.enter_context(tile.TileContext(nc))
        in_pool = ctx.enter_context(tc.tile_pool(name="inp", bufs=2))
        psum_pool = ctx.enter_context(tc.tile_pool(name="psum", bufs=2, space="PSUM"))
        d_pool = ctx.enter_context(tc.tile_pool(name="d16", bufs=3))
        scr_pool = ctx.enter_context(tc.tile_pool(name="scr", bufs=2))
        acc_pool = ctx.enter_context(tc.tile_pool(name="acc", bufs=3))
        rm_pool = ctx.enter_context(tc.tile_pool(name="rm", bufs=2))

        for b in range(BPC):
            lr_sb = in_pool.tile([K, 2 * N], bf16, tag="lr")
            nc.sync.dma_start(lr_sb[:], lr[b])
            rm = rm_pool.tile([128, NT], f32, tag="rm")
            acc = None
            for nt in range(NT):
                ps = psum_pool.tile([128, N], f32, tag="ps")
                for mc in range(MC):
                    nc.tensor.matmul(
                        ps[:, mc * MCH : (mc + 1) * MCH],
                        lr_sb[:, nt * 128 : (nt + 1) * 128],
                        lr_sb[:, N + mc * MCH : N + (mc + 1) * MCH],
                        start=True,
                        stop=True,
                    )
                d16 = d_pool.tile([128, N], f16, tag="d16")
                nc.scalar.copy(d16[:], ps[:])
                # Row-min of d16 via a tensor_tensor min tree (2x fp16 mode;
                # tensor_reduce would be 1x) finished by one small reduce.
                half = d_pool.tile([128, N // 2], f16, tag="half", name="half")
                nc.vector.tensor_tensor(half[:], d16[:, : N // 2], d16[:, N // 2 :], amin)
                quart = d_pool.tile([128, N // 4], f16, tag="quart", name="quart")
                nc.vector.tensor_tensor(
                    quart[:], half[:, : N // 4], half[:, N // 4 :], amin
                )
                eighth = d_pool.tile([128, N // 8], f16, tag="eighth", name="eighth")
                nc.vector.tensor_tensor(
                    eighth[:], quart[:, : N // 8], quart[:, N // 8 :], amin
                )
                nc.vector.tensor_reduce(
                    rm[:, nt : nt + 1], eighth[:], axis=mybir.AxisListType.X, op=amin
                )
                # Running column-min across n-tiles (ping-pong, 2x fp16 mode).
                if nt == 0:
                    acc = acc_pool.tile([128, N], f16, tag="acc", name="acc0")
                    nc.vector.tensor_copy(acc[:], d16[:])
                else:
                    acc_new = acc_pool.tile([128, N], f16, tag="acc")
                    nc.vector.tensor_tensor(acc_new[:], acc[:], d16[:], amin)
                    acc = acc_new
            nc.sync.dma_start(rowmins[b], rm[:])
            nc.sync.dma_start(colmin[b], acc[:])
    nc.compile()
    return nc


def _get_program():
    if "nc" not in _CACHE:
        _CACHE["nc"] = _build_program()
    return _CACHE["nc"]


def _host_prep(output_set, output_mask, target_set, target_mask):
    """Build the K=16 bf16 hi/lo-split gram-trick operands.

    d[n,m] = x2[n] + y2[m] - 2*x.y, with every term expressed as a sum of
    exact bf16 x bf16 products (accumulated in fp32 PSUM):
      rows 0-2 : (-2*ax_c) * ay_c     rows 3-5 : (-2*ax_c) * by_c
      rows 6-8 : (-2*bx_c) * ay_c     rows 9-11: (-2*bx_c) * by_c
      row 12: x2h*1   row 13: x2l*1   row 14: 1*y2h   row 15: 1*y2l
    where ax=bf16(x), bx=bf16(x-ax) (residual ~2^-16), same for y/x2/y2.
    """
    import ml_dtypes

    bf = ml_dtypes.bfloat16
    maxabs = max(
        np.abs(output_set[~output_mask]).max(initial=0.0),
        np.abs(target_set[~target_mask]).max(initial=0.0),
    )
    assert maxabs <= 14.0, f"coordinate magnitude {maxabs} too large for fp16 fill"
    x = np.where(output_mask[..., None], np.float32(FILL), output_set.astype(np.float32))
    y = np.where(target_mask[..., None], np.float32(-FILL), target_set.astype(np.float32))
    x2 = np.sum(x * x, axis=-1, dtype=np.float32)  # [B, N]
    y2 = np.sum(y * y, axis=-1, dtype=np.float32)

    def split(v):
        hi = v.astype(bf)
        lo = (v - hi.astype(np.float32)).astype(bf)
        return hi.astype(np.float32), lo.astype(np.float32)

    ax, bx = split(x)  # [B, N, 3] each
    ay, by = split(y)
    x2h, x2l = split(x2)  # [B, N]
    y2h, y2l = split(y2)
    ones = np.ones_like(x2)
    zeros = np.zeros_like(x2)

    def rows(*rs):
        return np.stack(rs, axis=1)  # [B, K, N]

    lx = rows(
        *(-2.0 * ax[..., c] for c in range(3)),
        *(-2.0 * ax[..., c] for c in range(3)),
        *(-2.0 * bx[..., c] for c in range(3)),
        *(-2.0 * bx[..., c] for c in range(3)),
        x2h, x2l, ones, ones,
    )
    ry = rows(
        *(ay[..., c] for c in range(3)),
        *(by[..., c] for c in range(3)),
        *(ay[..., c] for c in range(3)),
        *(by[..., c] for c in range(3)),
        ones, ones, y2h, y2l,
    )
    lr = np.concatenate([lx, ry], axis=2)  # [B, K, 2N]
    # -2*ax etc. are exact in bf16 (sign/exponent ops); x2h/ones exact too.
    return np.ascontiguousarray(lr.astype(bf))


def _finalize(rowmins, colmin, output_mask, target_mask, kls):
    # rowmins: [B, 128, NT] f32 with min1[n = nt*128 + p] = rowmins[b, p, nt]
    min1 = rowmins.transpose(0, 2, 1).reshape(B, N).astype(np.float64)
    min2 = colmin.astype(np.float32).min(axis=1).astype(np.float64)  # [B, N]
    validx = ~output_mask
    validy = ~target_mask
    nx = validx.sum(axis=1).astype(np.float64)
    ny = validy.sum(axis=1).astype(np.float64)
    s1 = np.where(validx, min1, 0.0).sum(axis=1)
    s2 = np.where(validy, min2, 0.0).sum(axis=1)
    with np.errstate(divide="ignore", invalid="ignore"):
        per_sample = s1 / nx + s2 / ny
    l2_loss = np.float32(per_sample.mean())

    kl64 = kls.astype(np.float64)
    kl_loss = np.float32(kl64.sum(axis=1).mean())
    loss = np.float32(np.float32(BETA) * kl_loss + l2_loss)
    topdown_kl = (kl64.mean(axis=0) / (Z_SCALES.astype(np.float64) * Z_DIM)).astype(
        np.float32
    )
    beta = np.float32(BETA)
    return loss, kl_loss, l2_loss, topdown_kl, beta


def _run_device(lr, trace=False):
    import concourse.bass_utils as bass_utils

    nc = _get_program()
    in_maps = [{"lr": lr[c * BPC : (c + 1) * BPC]} for c in range(NCORES)]
    res = bass_utils.run_bass_kernel_spmd(
        nc, in_maps, core_ids=list(range(NCORES)), trace=trace
    )
    rowmins = np.concatenate([r["rowmins"] for r in res.results], axis=0)
    colmin = np.concatenate([r["colmin"] for r in res.results], axis=0)
    return rowmins.reshape(B, 128, NT), colmin.reshape(B, 128, N), res


def kernel(output_set, output_mask, target_set, target_mask, kls):
    output_set = np.asarray(output_set)
    output_mask = np.asarray(output_mask).astype(bool)
    target_set = np.asarray(target_set)
    target_mask = np.asarray(target_mask).astype(bool)
    kls = np.asarray(kls)
    assert output_set.shape == (B, N, D), output_set.shape
    lr = _host_prep(output_set, output_mask, target_set, target_mask)
    rowmins, colmin, _ = _run_device(lr)
    return _finalize(rowmins, colmin, output_mask, target_mask, kls)
